# revision 36
# baseline (speedup 1.0000x reference)
"""AdmissibleStatesHead on 8 Trainium2 NeuronCores.

marginals[c] = segment_sum(softmax(E @ W.T + b), digit_c)  ==  P @ M_c
where M is a one-hot [N_VALID, 48] matrix built on host from valid_states.

Device work per core (valid-states sharded 8 ways, batch replicated):
  logits^T tile [128v, 512b] = sum_k wt[k,v].T @ et[k,b]   (fp8 DoubleRow PE, fp32 PSUM)
  exp tile = Exp(logits^T * inv_scale + bias)              (ScalarE, PSUM -> SBUF fp8)
  U^T [48, 512b] += M_chunk.T @ exp_tile                   (fp8 DoubleRow PE)
Host: sum per-core partials, normalize by concept-0 bucket sum (= softmax
denominator), reshape to [6, B, 8]. W is pre-scaled by a power of two into
fp8's range; the Exp activation's free affine undoes it.

Set KERNEL_BF16=1 for a bf16 fallback (~1.8x slower, ~15x more accurate);
KERNEL_TRACE=1 captures an NTFF profile and fills LAST_EXEC_NS.
"""

import os
import sys
import types

import numpy as np
import ml_dtypes

OUTCOMES = [8, 8, 8, 8, 8, 8]
N_TOTAL = 262144
N_VALID = 8192
B, D = 4096, 1024
N_CORES = 8
P = 128
V_S = N_VALID // N_CORES  # 1024 valid states per core
NK = D // P               # 8 contraction chunks
NV = V_S // P             # 8 v-tiles per core
NB = B // 512             # 8 batch tiles of 512
NJ = 48                   # 6 concepts x 8 outcomes

# W values are small (~N(0, 0.02^2) per spec); scale into fp8e4m3's normal
# range and undo the scale for free inside the Exp activation. Chosen per
# call from the data as a power of two; the compiled module is cached per
# scale value.
DEFAULT_W_SCALE = 64.0

USE_BF16 = bool(os.environ.get("KERNEL_BF16"))

LAST_EXEC_NS = None
LAST_RESULT = None
_compiled_cache = {}


def _pick_w_scale(wmax):
    import math

    if not np.isfinite(wmax) or wmax <= 0:
        return DEFAULT_W_SCALE
    # keep max|W*scale| around <=192 (fp8e4m3 max 448), scale a power of 2
    s = 2.0 ** math.floor(math.log2(192.0 / wmax))
    return float(min(max(s, 2.0 ** -10), 2.0 ** 20))


def _split_excess_waits(nc, limit=1):
    """This walrus build rejects instructions carrying more than ~1 sync-wait
    ("Too many sync wait commands"). Hoist excess waits onto injected NoOps
    right before the instruction on the same engine — sequencers are in-order,
    so the semantics are identical."""
    import concourse.mybir as mybir

    ctr = 0
    main_bb = nc.m.functions[0].blocks[0]
    stripped = []
    for ins in main_bb.instructions:
        nm = str(ins.name)
        op = ins.concise_opcode()
        if op == "Drain" or (op == "EventSemaphore" and nm.startswith("barrier_")):
            continue
        stripped.append(ins)
    main_bb.instructions = stripped
    for fn in nc.m.functions:
        for bb in fn.blocks:
            insts = bb.instructions
            new = []
            changed = False
            for ins in insts:
                si = ins.sync_info
                lim = 1 if ins.concise_opcode() == "Drain" else limit
                if si is not None and len(si.on_wait) > lim:
                    waits = list(si.on_wait)
                    for w in waits[:-lim]:
                        ctr += 1
                        nop = mybir.InstNoOp(name=f"waitsplit_{ctr}", ins=[], outs=[])
                        nop.engine = ins.engine
                        nop.sync_info = mybir.SyncInfo(on_update=[], on_wait=[w])
                        new.append(nop)
                    ins.sync_info = mybir.SyncInfo(
                        on_update=list(si.on_update), on_wait=waits[-lim:]
                    )
                    changed = True
                new.append(ins)
            if changed:
                bb.instructions = new


def _patch_tile_tail():
    import concourse.tile as tile
    from concourse.vector_clock import ScopedClock

    if getattr(tile.TileContext, "_tail_patched", False):
        return

    def _drain_and_barrier(self, tick_clock, wait_clock):
        drain_inst = self.nc.sync.drain()
        wait_clock.add_sem_waits(
            drain_inst.ins, ScopedClock({None: tick_clock.global_clock})
        )
        self.nc.all_engine_barrier()
        popped = self.nc._tile_sem_poison_stack.pop()
        assert popped is self._sem_poison
        self.nc.clear_and_free_semaphores(list(self.sems.allocated().values()))

    tile.TileContext._drain_and_barrier = _drain_and_barrier
    tile.TileContext._tail_patched = True


def _build_nc(w_scale):
    import concourse.bass as bass
    import concourse.mybir as mybir
    import concourse.tile as tile

    _patch_tile_tail()

    f32 = mybir.dt.float32
    bf16 = mybir.dt.bfloat16
    fp8 = mybir.dt.float8e4
    Exp = mybir.ActivationFunctionType.Exp

    in_dt = bf16 if USE_BF16 else fp8
    exp_scale = 1.0 if USE_BF16 else 1.0 / w_scale

    nc = bass.Bass()
    wt = nc.dram_tensor("wt", [P, NV, NK, P], in_dt, kind="ExternalInput")
    et = nc.dram_tensor("et", [NB, P, NK, 512], in_dt, kind="ExternalInput")
    mm = nc.dram_tensor("mm", [P, NV, NJ], in_dt, kind="ExternalInput")
    bias = nc.dram_tensor("bias", [P, NV], f32, kind="ExternalInput")
    out = nc.dram_tensor("out", [NJ, NB, 512], f32, kind="ExternalOutput")

    with (
        tile.TileContext(nc) as tc,
        tc.tile_pool(name="const", bufs=1) as cpool,
        tc.tile_pool(name="etp", bufs=8) as epool,
        tc.tile_pool(name="expp", bufs=3) as xpool,
        tc.tile_pool(name="ps", bufs=5, space="PSUM") as pspool,
        tc.tile_pool(name="ps2", bufs=2, space="PSUM") as ps2pool,
        tc.tile_pool(name="uo", bufs=2) as upool,
        tc.tile_pool(name="warm", bufs=1) as wpool,
        tc.tile_pool(name="warmps", bufs=1, space="PSUM") as wpspool,
    ):
        # PE HAM warm-up: the clock gate only opens after ~3.4us of sustained
        # PE activity. The input DMAs take ~4us, so run throwaway matmuls on
        # zeroed scratch during that window; the real matmul stream then
        # starts at 2.4 GHz instead of 1.2.
        warm_sb = wpool.tile([P, 512], in_dt)
        nc.gpsimd.memset(warm_sb[:], 0)
        warm_ps = wpspool.tile([P, 512], f32)
        for _ in range(10):
            nc.tensor.matmul(
                warm_ps[:],
                lhsT=warm_sb[:, 0:P],
                rhs=warm_sb[:],
                start=True,
                stop=True,
            )
        # Issue order matters: the SP HWDGE ring is FIFO (triggers ~0.6us
        # each, data roughly in trigger order) and the first matmul needs
        # et tile 0 + wt chunk 0 — front-load those, consolidate the rest.
        # n=0's et is split into two tiles so the first matmul only waits
        # for the first k-pair (128KB) plus wt chunk 0.
        wt_sb = cpool.tile([P, NV, NK, P], in_dt)
        nc.sync.dma_start(wt_sb[:, 0], wt[:, 0])
        et0a = cpool.tile([P, 2, 512], in_dt)
        nc.sync.dma_start(et0a[:], et[0][:, 0:2])
        et0b = cpool.tile([P, 6, 512], in_dt)
        nc.sync.dma_start(et0b[:], et[0][:, 2:])
        et_tiles = [(et0a, et0b)]
        nc.sync.dma_start(wt_sb[:, 1], wt[:, 1])
        nc.sync.dma_start(wt_sb[:, 2:4], wt[:, 2:4])
        nc.sync.dma_start(wt_sb[:, 4:6], wt[:, 4:6])
        nc.sync.dma_start(wt_sb[:, 6:8], wt[:, 6:8])
        m_sb = cpool.tile([P, NV, NJ], in_dt)
        nc.sync.dma_start(m_sb[:], mm[:])
        b_sb = cpool.tile([P, NV], f32)
        nc.sync.dma_start(b_sb[:], bias[:])
        for n in range(1, NB):
            t = epool.tile([P, NK, 512], in_dt, tag="et", name="et_t")
            nc.sync.dma_start(t[:], et[n])
            et_tiles.append((t[:, 0:2], t[:, 2:]))

        def emit_mm2(n, exp_t):
            if USE_BF16:
                ups = ps2pool.tile([NJ, 512], f32, tag="ups")
                for v in range(NV):
                    nc.tensor.matmul(
                        ups[:],
                        lhsT=m_sb[:, v, :],
                        rhs=exp_t[:, v, :],
                        start=(v == 0),
                        stop=(v == NV - 1),
                    )
                u_sb = upool.tile([NJ, 512], f32, tag="u")
                nc.vector.tensor_copy(u_sb[:], ups[:])
            else:
                ups = ps2pool.tile([NJ, 512], f32, tag="ups")
                for v in range(0, NV, 2):
                    nc.tensor.matmul(
                        ups[:],
                        lhsT=m_sb[:, v : v + 2, :],
                        rhs=exp_t[:, v : v + 2, :],
                        start=(v == 0),
                        stop=(v == NV - 2),
                        perf_mode=mybir.MatmulPerfMode.DoubleRow,
                    )
                u_sb = upool.tile([NJ, 512], f32, tag="u")
                nc.vector.tensor_copy(u_sb[:], ups[:])
            nc.sync.dma_start(out[:, n, :], u_sb[:])

        pending = None  # (n, exp_t) whose MM2 is deferred one tile
        for n in range(NB):
            et_a, et_b = et_tiles[n]
            exp_t = xpool.tile([P, NV, 512], in_dt, tag="exp")
            for v in range(NV):
                ps = pspool.tile([P, 512], f32, tag="ps")
                if USE_BF16:
                    for k in range(NK):
                        rhs = et_a[:, k, :] if k < 2 else et_b[:, k - 2, :]
                        nc.tensor.matmul(
                            ps[:],
                            lhsT=wt_sb[:, v, k],
                            rhs=rhs,
                            start=(k == 0),
                            stop=(k == NK - 1),
                        )
                else:
                    for k in range(0, NK, 2):
                        rhs = et_a[:] if k == 0 else et_b[:, k - 2 : k, :]
                        nc.tensor.matmul(
                            ps[:],
                            lhsT=wt_sb[:, v, k : k + 2],
                            rhs=rhs,
                            start=(k == 0),
                            stop=(k == NK - 2),
                            perf_mode=mybir.MatmulPerfMode.DoubleRow,
                        )
                nc.scalar.activation(
                    exp_t[:, v, :], ps[:], Exp, bias=b_sb[:, v : v + 1], scale=exp_scale
                )
                if pending is not None and v == 1:
                    emit_mm2(*pending)
                    pending = None
            pending = (n, exp_t)
        emit_mm2(*pending)
    _split_excess_waits(nc)
    return nc


def _install_ntff_hook():
    """bass_utils' axon trace path imports antenv.axon_hooks, absent in this
    image; shim it using trn_boot's ctypes NTFF hook."""
    if "antenv.axon_hooks" in sys.modules:
        return
    try:
        from trn_agent_boot.trn_boot import _ntff_profile_via_ctypes

        hook = _ntff_profile_via_ctypes("/opt/axon/libaxon_pjrt.so")
    except Exception:
        hook = None
    mod = types.ModuleType("antenv.axon_hooks")
    mod.get_axon_ntff_profile_hook = lambda: hook
    sys.modules["antenv.axon_hooks"] = mod


def kernel(embeddings, W, b, valid_states):
    global LAST_EXEC_NS, LAST_RESULT
    E = np.asarray(embeddings, dtype=np.float32)
    Wf = np.asarray(W, dtype=np.float32)
    bf = np.asarray(b, dtype=np.float32)
    vs = np.asarray(valid_states).astype(np.int64)

    bf16 = ml_dtypes.bfloat16
    if USE_BF16:
        in_dt = bf16
        Wp = Wf
        w_scale = 1.0
    else:
        in_dt = ml_dtypes.float8_e4m3
        w_scale = _pick_w_scale(float(np.abs(Wf).max()))
        Wp = Wf * w_scale

    # et[n, p, k, j] = E[n*512+j, k*128+p]
    Et = E.T.astype(in_dt)  # [D, B]
    et_host = np.ascontiguousarray(Et.reshape(NK, P, NB, 512).transpose(2, 1, 0, 3))

    # One-hot segment matrix M [N_VALID, 48]
    M = np.zeros((N_VALID, NJ), dtype=in_dt)
    stride = N_TOTAL
    for c, n_i in enumerate(OUTCOMES):
        stride //= n_i
        digit = (vs // stride) % n_i
        M[np.arange(N_VALID), c * 8 + digit] = 1

    in_maps = []
    for core in range(N_CORES):
        sl = slice(core * V_S, (core + 1) * V_S)
        wt_host = np.ascontiguousarray(
            Wp[sl, :].T.astype(in_dt).reshape(NK, P, NV, P).transpose(1, 2, 0, 3)
        )
        m_host = np.ascontiguousarray(M[sl].reshape(NV, P, NJ).transpose(1, 0, 2))
        b_host = np.ascontiguousarray(bf[sl].reshape(NV, P).T)
        in_maps.append({"wt": wt_host, "et": et_host, "mm": m_host, "bias": b_host})

    from concourse.bass_utils import run_bass_kernel_spmd

    key = (USE_BF16, w_scale)
    if key not in _compiled_cache:
        _compiled_cache[key] = _build_nc(w_scale)
    nc_mod = _compiled_cache[key]

    kwargs = {}
    if os.environ.get("KERNEL_TRACE"):
        _install_ntff_hook()
        kwargs["trace"] = True

    res = run_bass_kernel_spmd(
        nc_mod, in_maps, core_ids=list(range(N_CORES)), **kwargs
    )
    LAST_EXEC_NS = res.exec_time_ns
    LAST_RESULT = res

    U = np.zeros((NJ, B), dtype=np.float64)
    for r in res.results:
        U += r["out"].reshape(NJ, B).astype(np.float64)
    denom = U[0:8].sum(axis=0)  # [B] total softmax denominator
    marg = U.reshape(6, 8, B) / denom  # [6, 8, B]
    return np.ascontiguousarray(marg.transpose(0, 2, 1)).astype(np.float32)


# revision 37
# speedup vs baseline: 1.0061x; 1.0061x over previous
"""AdmissibleStatesHead on 8 Trainium2 NeuronCores.

marginals[c] = segment_sum(softmax(E @ W.T + b), digit_c)  ==  P @ M_c
where M is a one-hot [N_VALID, 48] matrix built on host from valid_states.

Device work per core (valid-states sharded 8 ways, batch replicated):
  logits^T tile [128v, 512b] = sum_k wt[k,v].T @ et[k,b]   (fp8 DoubleRow PE, fp32 PSUM)
  exp tile = Exp(logits^T * inv_scale + bias)              (ScalarE, PSUM -> SBUF fp8)
  U^T [48, 512b] += M_chunk.T @ exp_tile                   (fp8 DoubleRow PE)
Host: sum per-core partials, normalize by concept-0 bucket sum (= softmax
denominator), reshape to [6, B, 8]. W is pre-scaled by a power of two into
fp8's range; the Exp activation's free affine undoes it.

Set KERNEL_BF16=1 for a bf16 fallback (~1.8x slower, ~15x more accurate);
KERNEL_TRACE=1 captures an NTFF profile and fills LAST_EXEC_NS.
"""

import os
import sys
import types

import numpy as np
import ml_dtypes

OUTCOMES = [8, 8, 8, 8, 8, 8]
N_TOTAL = 262144
N_VALID = 8192
B, D = 4096, 1024
N_CORES = 8
P = 128
V_S = N_VALID // N_CORES  # 1024 valid states per core
NK = D // P               # 8 contraction chunks
NV = V_S // P             # 8 v-tiles per core
NB = B // 512             # 8 batch tiles of 512
NJ = 48                   # 6 concepts x 8 outcomes

# W values are small (~N(0, 0.02^2) per spec); scale into fp8e4m3's normal
# range and undo the scale for free inside the Exp activation. Chosen per
# call from the data as a power of two; the compiled module is cached per
# scale value.
DEFAULT_W_SCALE = 64.0

USE_BF16 = bool(os.environ.get("KERNEL_BF16"))

LAST_EXEC_NS = None
LAST_RESULT = None
_compiled_cache = {}


def _pick_w_scale(wmax):
    import math

    if not np.isfinite(wmax) or wmax <= 0:
        return DEFAULT_W_SCALE
    # keep max|W*scale| around <=192 (fp8e4m3 max 448), scale a power of 2
    s = 2.0 ** math.floor(math.log2(192.0 / wmax))
    return float(min(max(s, 2.0 ** -10), 2.0 ** 20))


def _split_excess_waits(nc, limit=1):
    """This walrus build rejects instructions carrying more than ~1 sync-wait
    ("Too many sync wait commands"). Hoist excess waits onto injected NoOps
    right before the instruction on the same engine — sequencers are in-order,
    so the semantics are identical."""
    import concourse.mybir as mybir

    ctr = 0
    main_bb = nc.m.functions[0].blocks[0]
    stripped = []
    for ins in main_bb.instructions:
        nm = str(ins.name)
        op = ins.concise_opcode()
        if op == "Drain" or (op == "EventSemaphore" and nm.startswith("barrier_")):
            continue
        stripped.append(ins)
    main_bb.instructions = stripped
    for fn in nc.m.functions:
        for bb in fn.blocks:
            insts = bb.instructions
            new = []
            changed = False
            for ins in insts:
                si = ins.sync_info
                lim = 1 if ins.concise_opcode() == "Drain" else limit
                if si is not None and len(si.on_wait) > lim:
                    waits = list(si.on_wait)
                    for w in waits[:-lim]:
                        ctr += 1
                        nop = mybir.InstNoOp(name=f"waitsplit_{ctr}", ins=[], outs=[])
                        nop.engine = ins.engine
                        nop.sync_info = mybir.SyncInfo(on_update=[], on_wait=[w])
                        new.append(nop)
                    ins.sync_info = mybir.SyncInfo(
                        on_update=list(si.on_update), on_wait=waits[-lim:]
                    )
                    changed = True
                new.append(ins)
            if changed:
                bb.instructions = new


def _patch_tile_tail():
    import concourse.tile as tile
    from concourse.vector_clock import ScopedClock

    if getattr(tile.TileContext, "_tail_patched", False):
        return

    def _drain_and_barrier(self, tick_clock, wait_clock):
        drain_inst = self.nc.sync.drain()
        wait_clock.add_sem_waits(
            drain_inst.ins, ScopedClock({None: tick_clock.global_clock})
        )
        self.nc.all_engine_barrier()
        popped = self.nc._tile_sem_poison_stack.pop()
        assert popped is self._sem_poison
        self.nc.clear_and_free_semaphores(list(self.sems.allocated().values()))

    tile.TileContext._drain_and_barrier = _drain_and_barrier
    tile.TileContext._tail_patched = True


def _build_nc(w_scale):
    import concourse.bass as bass
    import concourse.mybir as mybir
    import concourse.tile as tile

    _patch_tile_tail()

    f32 = mybir.dt.float32
    bf16 = mybir.dt.bfloat16
    fp8 = mybir.dt.float8e4
    Exp = mybir.ActivationFunctionType.Exp

    in_dt = bf16 if USE_BF16 else fp8
    exp_scale = 1.0 if USE_BF16 else 1.0 / w_scale

    nc = bass.Bass()
    wt = nc.dram_tensor("wt", [P, NV, NK, P], in_dt, kind="ExternalInput")
    et = nc.dram_tensor("et", [NB, P, NK, 512], in_dt, kind="ExternalInput")
    mm = nc.dram_tensor("mm", [P, NV, NJ], in_dt, kind="ExternalInput")
    bias = nc.dram_tensor("bias", [P, NV], f32, kind="ExternalInput")
    out = nc.dram_tensor("out", [NJ, NB, 512], f32, kind="ExternalOutput")

    with (
        tile.TileContext(nc) as tc,
        tc.tile_pool(name="const", bufs=1) as cpool,
        tc.tile_pool(name="etp", bufs=8) as epool,
        tc.tile_pool(name="expp", bufs=3) as xpool,
        tc.tile_pool(name="ps", bufs=5, space="PSUM") as pspool,
        tc.tile_pool(name="ps2", bufs=2, space="PSUM") as ps2pool,
        tc.tile_pool(name="uo", bufs=2) as upool,
        tc.tile_pool(name="warm", bufs=1) as wpool,
        tc.tile_pool(name="warmps", bufs=1, space="PSUM") as wpspool,
    ):
        # PE HAM warm-up: the clock gate only opens after ~3.4us of sustained
        # PE activity. The input DMAs take ~4us, so run throwaway matmuls on
        # zeroed scratch during that window; the real matmul stream then
        # starts at 2.4 GHz instead of 1.2.
        warm_sb = wpool.tile([P, 512], in_dt)
        nc.gpsimd.memset(warm_sb[:], 0)
        warm_act = wpool.tile([P, 16], in_dt)
        # pull the ~2.7us Exp table load into the DMA-wait window
        nc.scalar.activation(warm_act[:], warm_sb[:, 0:16], Exp)
        warm_ps = wpspool.tile([P, 512], f32)
        for _ in range(10):
            nc.tensor.matmul(
                warm_ps[:],
                lhsT=warm_sb[:, 0:P],
                rhs=warm_sb[:],
                start=True,
                stop=True,
            )
        # Issue order matters: the SP HWDGE ring is FIFO (triggers ~0.6us
        # each, data roughly in trigger order) and the first matmul needs
        # et tile 0 + wt chunk 0 — front-load those, consolidate the rest.
        # n=0's et is split into two tiles so the first matmul only waits
        # for the first k-pair (128KB) plus wt chunk 0.
        wt_sb = cpool.tile([P, NV, NK, P], in_dt)
        nc.sync.dma_start(wt_sb[:, 0], wt[:, 0])
        et0a = cpool.tile([P, 2, 512], in_dt)
        nc.sync.dma_start(et0a[:], et[0][:, 0:2])
        et0b = cpool.tile([P, 6, 512], in_dt)
        nc.sync.dma_start(et0b[:], et[0][:, 2:])
        et_tiles = [(et0a, et0b)]
        nc.sync.dma_start(wt_sb[:, 1], wt[:, 1])
        nc.sync.dma_start(wt_sb[:, 2:4], wt[:, 2:4])
        nc.sync.dma_start(wt_sb[:, 4:6], wt[:, 4:6])
        nc.sync.dma_start(wt_sb[:, 6:8], wt[:, 6:8])
        m_sb = cpool.tile([P, NV, NJ], in_dt)
        nc.sync.dma_start(m_sb[:], mm[:])
        b_sb = cpool.tile([P, NV], f32)
        nc.sync.dma_start(b_sb[:], bias[:])
        for n in range(1, NB):
            t = epool.tile([P, NK, 512], in_dt, tag="et", name="et_t")
            nc.sync.dma_start(t[:], et[n])
            et_tiles.append((t[:, 0:2], t[:, 2:]))

        def emit_mm2(n, exp_t):
            if USE_BF16:
                ups = ps2pool.tile([NJ, 512], f32, tag="ups")
                for v in range(NV):
                    nc.tensor.matmul(
                        ups[:],
                        lhsT=m_sb[:, v, :],
                        rhs=exp_t[:, v, :],
                        start=(v == 0),
                        stop=(v == NV - 1),
                    )
                u_sb = upool.tile([NJ, 512], f32, tag="u")
                nc.vector.tensor_copy(u_sb[:], ups[:])
            else:
                ups = ps2pool.tile([NJ, 512], f32, tag="ups")
                for v in range(0, NV, 2):
                    nc.tensor.matmul(
                        ups[:],
                        lhsT=m_sb[:, v : v + 2, :],
                        rhs=exp_t[:, v : v + 2, :],
                        start=(v == 0),
                        stop=(v == NV - 2),
                        perf_mode=mybir.MatmulPerfMode.DoubleRow,
                    )
                u_sb = upool.tile([NJ, 512], f32, tag="u")
                nc.vector.tensor_copy(u_sb[:], ups[:])
            nc.sync.dma_start(out[:, n, :], u_sb[:])

        pending = None  # (n, exp_t) whose MM2 is deferred one tile
        for n in range(NB):
            et_a, et_b = et_tiles[n]
            exp_t = xpool.tile([P, NV, 512], in_dt, tag="exp")
            for v in range(NV):
                ps = pspool.tile([P, 512], f32, tag="ps")
                if USE_BF16:
                    for k in range(NK):
                        rhs = et_a[:, k, :] if k < 2 else et_b[:, k - 2, :]
                        nc.tensor.matmul(
                            ps[:],
                            lhsT=wt_sb[:, v, k],
                            rhs=rhs,
                            start=(k == 0),
                            stop=(k == NK - 1),
                        )
                else:
                    for k in range(0, NK, 2):
                        rhs = et_a[:] if k == 0 else et_b[:, k - 2 : k, :]
                        nc.tensor.matmul(
                            ps[:],
                            lhsT=wt_sb[:, v, k : k + 2],
                            rhs=rhs,
                            start=(k == 0),
                            stop=(k == NK - 2),
                            perf_mode=mybir.MatmulPerfMode.DoubleRow,
                        )
                nc.scalar.activation(
                    exp_t[:, v, :], ps[:], Exp, bias=b_sb[:, v : v + 1], scale=exp_scale
                )
                if pending is not None and v == 1:
                    emit_mm2(*pending)
                    pending = None
            pending = (n, exp_t)
        emit_mm2(*pending)
    _split_excess_waits(nc)
    return nc


def _install_ntff_hook():
    """bass_utils' axon trace path imports antenv.axon_hooks, absent in this
    image; shim it using trn_boot's ctypes NTFF hook."""
    if "antenv.axon_hooks" in sys.modules:
        return
    try:
        from trn_agent_boot.trn_boot import _ntff_profile_via_ctypes

        hook = _ntff_profile_via_ctypes("/opt/axon/libaxon_pjrt.so")
    except Exception:
        hook = None
    mod = types.ModuleType("antenv.axon_hooks")
    mod.get_axon_ntff_profile_hook = lambda: hook
    sys.modules["antenv.axon_hooks"] = mod


def kernel(embeddings, W, b, valid_states):
    global LAST_EXEC_NS, LAST_RESULT
    E = np.asarray(embeddings, dtype=np.float32)
    Wf = np.asarray(W, dtype=np.float32)
    bf = np.asarray(b, dtype=np.float32)
    vs = np.asarray(valid_states).astype(np.int64)

    bf16 = ml_dtypes.bfloat16
    if USE_BF16:
        in_dt = bf16
        Wp = Wf
        w_scale = 1.0
    else:
        in_dt = ml_dtypes.float8_e4m3
        w_scale = _pick_w_scale(float(np.abs(Wf).max()))
        Wp = Wf * w_scale

    # et[n, p, k, j] = E[n*512+j, k*128+p]
    Et = E.T.astype(in_dt)  # [D, B]
    et_host = np.ascontiguousarray(Et.reshape(NK, P, NB, 512).transpose(2, 1, 0, 3))

    # One-hot segment matrix M [N_VALID, 48]
    M = np.zeros((N_VALID, NJ), dtype=in_dt)
    stride = N_TOTAL
    for c, n_i in enumerate(OUTCOMES):
        stride //= n_i
        digit = (vs // stride) % n_i
        M[np.arange(N_VALID), c * 8 + digit] = 1

    in_maps = []
    for core in range(N_CORES):
        sl = slice(core * V_S, (core + 1) * V_S)
        wt_host = np.ascontiguousarray(
            Wp[sl, :].T.astype(in_dt).reshape(NK, P, NV, P).transpose(1, 2, 0, 3)
        )
        m_host = np.ascontiguousarray(M[sl].reshape(NV, P, NJ).transpose(1, 0, 2))
        b_host = np.ascontiguousarray(bf[sl].reshape(NV, P).T)
        in_maps.append({"wt": wt_host, "et": et_host, "mm": m_host, "bias": b_host})

    from concourse.bass_utils import run_bass_kernel_spmd

    key = (USE_BF16, w_scale)
    if key not in _compiled_cache:
        _compiled_cache[key] = _build_nc(w_scale)
    nc_mod = _compiled_cache[key]

    kwargs = {}
    if os.environ.get("KERNEL_TRACE"):
        _install_ntff_hook()
        kwargs["trace"] = True

    res = run_bass_kernel_spmd(
        nc_mod, in_maps, core_ids=list(range(N_CORES)), **kwargs
    )
    LAST_EXEC_NS = res.exec_time_ns
    LAST_RESULT = res

    U = np.zeros((NJ, B), dtype=np.float64)
    for r in res.results:
        U += r["out"].reshape(NJ, B).astype(np.float64)
    denom = U[0:8].sum(axis=0)  # [B] total softmax denominator
    marg = U.reshape(6, 8, B) / denom  # [6, 8, B]
    return np.ascontiguousarray(marg.transpose(0, 2, 1)).astype(np.float32)
